# revision 1
# baseline (speedup 1.0000x reference)
"""Trainium2 Bass kernel for nn_Attention (channel-attention, 8 NeuronCores).

Algorithm (algebraically identical to the reference):
  The attention contracts over the spatial axis n = 32*32*32 = 32768, and the
  attention matrices are tiny (64x64 per head).  Everything collapses around
  the per-batch Gram matrix G_b = x_b @ x_b^T (128x128):

    scores_bh = scale * Wq_h G_b Wk_h^T            (tiny)
    attn      = softmax(scores)                     (tiny)
    W_eff_b   = (1/n) * sum_h Wout_h attn_bh Wv_h   (64x128, tiny)
    y_b       = W_eff_b @ x_b + b_out               (the only other big matmul)

  Sharding: spatial n split across the 8 cores (4096 each).  Each core
  computes a partial G over its shard (the only big contraction), a 64KB
  bf16 AllReduce combines them, the tiny attention algebra is replicated on
  every core, and each core produces its own n-slice of y.  All matmuls run
  in bf16 with f32 accumulation (rounding errors average out over the huge
  contractions; measured end-to-end max rel err ~1e-4).

  x is shipped once, in [n, c] layout (needed by the Gram matmuls); the
  [c, n] layout needed by the final y matmul is produced on-chip with PE
  transposes scheduled under the AllReduce wait, which also keeps the PE
  HAM-warm through the collective.
"""

import numpy as np
import ml_dtypes

import concourse.bass as bass
import concourse.bacc as bacc
import concourse.mybir as mybir
import concourse.tile as tile
from concourse.tile import add_dep_helper
from concourse.bass_utils import run_bass_kernel_spmd

NCORES = 8
P = 128
N_TOT = 32 * 32 * 32          # 32768 spatial points
NSH = N_TOT // NCORES         # 4096 per core per batch
F = 2 * NSH                   # 8192 free columns (both batches side by side)
NCHUNK = 4                    # xn DMA chunks (pipelined with the G matmuls)
DUMMY_WARM_MMS = 38           # HAM warm-keeper fp32 matmuls under the AR wait
HEADS = 8
DH = 64
SCALE = DH ** -0.5
BF = mybir.dt.bfloat16
F32 = mybir.dt.float32
bf16 = ml_dtypes.bfloat16

_CACHED_NC = None


class _TrimmedTileContext(tile.TileContext):
    """TileContext minus the FINAL all-engine barrier of the exit sequence.

    The stock exit is drain -> barrier -> sem-clear -> barrier; the last
    barrier only makes every engine wait for the gpsimd sem-clear before
    halting, which matters for looped NEFFs but not a single-shot kernel:
    the clear still completes before its issuing engine halts, so a
    re-execution starts with zeroed semaphores either way.  Dropping it
    saves ~4us of measured EVSEM-butterfly tail.
    """

    def _drain_and_barrier(self, tick_clock, wait_clock):
        from concourse.vector_clock import ScopedClock

        drain_inst = self.nc.sync.drain()
        wait_clock.add_sem_waits(
            drain_inst.ins, ScopedClock({None: tick_clock.global_clock})
        )
        self.nc.all_engine_barrier()
        popped = self.nc._tile_sem_poison_stack.pop()
        assert popped is self._sem_poison
        self.nc.clear_and_free_semaphores(list(self.sems.allocated().values()))


def build_nc():
    nc = bacc.Bacc(
        "TRN2", target_bir_lowering=False, debug=False, num_devices=NCORES
    )

    xn_ext = nc.dram_tensor("xn", [P, F], BF, kind="ExternalInput")
    wq_ext = nc.dram_tensor("wqT", [P, 512], BF, kind="ExternalInput")
    wk_ext = nc.dram_tensor("wkT", [P, 512], BF, kind="ExternalInput")
    wv_ext = nc.dram_tensor("wv", [P, 512], BF, kind="ExternalInput")
    wo_ext = nc.dram_tensor("woT", [P, 256], BF, kind="ExternalInput")
    bo_ext = nc.dram_tensor("bout", [P, 1], F32, kind="ExternalInput")
    id_ext = nc.dram_tensor("ident", [P, P], BF, kind="ExternalInput")
    out_ext = nc.dram_tensor("out", [P, NSH], F32, kind="ExternalOutput")

    with _TrimmedTileContext(nc) as tc:
        with (
            tc.tile_pool(name="const", bufs=1) as const,
            tc.tile_pool(name="data", bufs=1) as data,
            tc.tile_pool(name="work", bufs=1) as work,
            tc.tile_pool(name="ypool", bufs=1) as ypool,
            tc.tile_pool(name="psg", bufs=2, space="PSUM") as psg,
            tc.tile_pool(name="psd", bufs=2, space="PSUM") as psd,
            tc.tile_pool(name="psy", bufs=2, space="PSUM") as psy,
            tc.tile_pool(name="dram", bufs=1, space="DRAM") as dram,
        ):
            # ---- ncfw warm-up: a tiny dependency-free AllReduce triggered
            # right after the preamble.  The real collective's doorbell is
            # otherwise noticed ~30-50us late (collective-firmware wakeup);
            # queueing this one first can absorb that latency.  Its own
            # completion is consumed by a throwaway DMA on an idle ring, and
            # since dummy-end (~trigger+stall+small mesh) always precedes the
            # real collective's own stall window, it can never delay it.
            warm_in = dram.tile([P, 2], F32, tag="warm_in")
            warm_out = dram.tile([P, 2], F32, tag="warm_out", addr_space="Shared")
            nc.gpsimd.collective_compute(
                "AllReduce",
                mybir.AluOpType.add,
                ins=[warm_in.opt()],
                outs=[warm_out.opt()],
                replica_groups=[[i] for i in range(NCORES)],
            )
            # ---- phase B: xn split across BOTH HWDGE rings; weights queue
            # behind the xn chunks (they are needed only much later) ----
            CH = F // NCHUNK  # 2048 columns (16 n-blocks) per chunk
            xn_tiles = []
            for c in range(NCHUNK):
                t = data.tile([P, CH], BF, tag=f"xn{c}")
                eng = nc.sync if c % 2 == 0 else nc.scalar
                eng.dma_start(t[:], xn_ext[:, c * CH : (c + 1) * CH])
                xn_tiles.append(t)

            wq = const.tile([P, 512], BF, tag="wq")
            nc.sync.dma_start(wq[:], wq_ext[:])
            wk = const.tile([P, 512], BF, tag="wk")
            nc.scalar.dma_start(wk[:], wk_ext[:])
            wv = const.tile([P, 512], BF, tag="wv")
            nc.sync.dma_start(wv[:], wv_ext[:])
            wo = const.tile([P, 256], BF, tag="wo")
            nc.scalar.dma_start(wo[:], wo_ext[:])
            bo = const.tile([P, 1], F32, tag="bo")
            nc.sync.dma_start(bo[:], bo_ext[:])
            ident = const.tile([P, P], BF, tag="ident")
            nc.scalar.dma_start(ident[:], id_ext[:])
            # warm-keeper source, zeroed early while the DVE is idle
            dummy_src = work.tile([P, 512], F32, tag="dummy")
            nc.vector.memset(dummy_src[:], 0.0)

            g_ps = [psg.tile([P, P], F32, tag="g", name=f"g_ps{b}") for b in range(2)]
            for c in range(NCHUNK):
                b = c // 2
                for tl in range(16):
                    gt = (c % 2) * 16 + tl  # accumulation index within batch
                    blk = xn_tiles[c][:, tl * P : (tl + 1) * P]
                    nc.tensor.matmul(
                        g_ps[b][:], blk, blk, start=(gt == 0), stop=(gt == 31)
                    )

            # bf16 partials -> 64KB AllReduce payload
            g_sb = work.tile([P, 256], BF, tag="gsb")
            for b in range(2):
                nc.vector.tensor_copy(g_sb[:, b * P : (b + 1) * P], g_ps[b][:])

            # ---- phase C: AllReduce the Gram over the 8 cores ----
            g_in = dram.tile([P, 256], BF, tag="gin")
            g_out = dram.tile([P, 256], BF, tag="gout", addr_space="Shared")
            g_dma = nc.sync.dma_start(g_in[:], g_sb[:])
            nc.gpsimd.collective_compute(
                "AllReduce",
                mybir.AluOpType.add,
                ins=[g_in.opt()],
                outs=[g_out.opt()],
                replica_groups=[list(range(NCORES))],
            )
            gbf = [
                work.tile([P, P], BF, tag=f"gbf{b}", name=f"gbf{b}")
                for b in range(2)
            ]
            for b in range(2):
                eng = nc.sync if b == 0 else nc.scalar
                eng.dma_start(gbf[b][:], g_out[:, b * P : (b + 1) * P])

            # ---- transpose xn -> xc in [c, n] layout, under the AR wait ----
            # Ordering-only deps on the G-path DMA keep the scheduler from
            # hoisting these ahead of the G matmuls (which would delay the
            # collective trigger); no runtime semaphore is added.
            xc = data.tile([P, F], BF, tag="xc")
            for c in range(NCHUNK):
                for tl in range(16):
                    col = c * CH + tl * P
                    tp = psy.tile([P, P], BF, tag="y", name=f"tp{c}_{tl}")
                    tri = nc.tensor.transpose(
                        tp[:], xn_tiles[c][:, tl * P : (tl + 1) * P], ident[:]
                    )
                    add_dep_helper(
                        tri.ins, g_dma.ins, sync=True,
                        reason="transposes ordered after the G path",
                    )
                    nc.vector.tensor_copy(xc[:, col : col + P], tp[:])

            # Dummy PE work to keep the HAM clock-gate warm through the
            # AllReduce wait so phases D/E run at 2.4 GHz, sized to roughly
            # the expected collective window.  fp32 matmuls run at 4
            # cycles/row (~850ns each), so few instructions cover a long
            # window.  Results are never read; the psum slots are the ones
            # the G partials released.
            last_warm = None
            for w in range(DUMMY_WARM_MMS):
                scratch = psg.tile([P, 512], F32, tag="g", name=f"warm{w}")
                wi = nc.tensor.matmul(
                    scratch[:], dummy_src[:, :P], dummy_src[:],
                    start=True, stop=True,
                )
                add_dep_helper(
                    wi.ins, g_dma.ins, sync=True,
                    reason="warm-keeper ordered after the G path",
                )
                last_warm = wi

            # ---- phase D: scores -> softmax -> W_eff (replicated, tiny) ----
            # scale folded into wqT on the host; 1/n folded into wv.
            # Batch 0/1 stages interleaved so the engines pipeline.
            sums = work.tile([P, 8], F32, tag="sums")
            recip = work.tile([P, 8], F32, tag="recip")
            weff = [
                work.tile([P, 64], BF, tag=f"weff{b}", name=f"weff{b}")
                for b in range(2)
            ]
            a_ps = [psd.tile([P, 512], F32, tag="d", name=f"a_ps{b}") for b in range(2)]
            a_sb = [work.tile([P, 512], BF, tag=f"asb{b}", name=f"a_sb{b}") for b in range(2)]
            s_ps = [psd.tile([P, 256], F32, tag="d", name=f"s_ps{b}") for b in range(2)]
            negmax = [work.tile([P, 4], F32, tag=f"nm{b}", name=f"negmax{b}") for b in range(2)]
            exp_sb = [work.tile([P, 256], F32, tag=f"exp{b}", name=f"exp_sb{b}") for b in range(2)]
            attn = [work.tile([P, 256], BF, tag=f"attn{b}", name=f"attn{b}") for b in range(2)]
            mt_ps = [psd.tile([P, 256], F32, tag="d2", name=f"mt_ps{b}") for b in range(2)]
            mt_sb = [work.tile([P, 256], BF, tag=f"mt{b}", name=f"mt_sb{b}") for b in range(2)]
            w_ps = [psd.tile([P, 64], F32, tag="d2", name=f"w_ps{b}") for b in range(2)]

            for b in range(2):
                ai = nc.tensor.matmul(
                    a_ps[b][:], gbf[b][:], wq[:],
                    start=True, stop=True,
                )
                if last_warm is not None:
                    add_dep_helper(
                        ai.ins, last_warm.ins, sync=False,
                        reason="phase D after the warm-keeper block",
                    )
            for b in range(2):
                # sliced so the first S matmuls start after slice 0 lands
                for sl in range(4):
                    nc.vector.tensor_copy(
                        a_sb[b][:, sl * 128 : (sl + 1) * 128],
                        a_ps[b][:, sl * 128 : (sl + 1) * 128],
                    )
            # S[i-half, j-group]: head h at partitions 64*(h%2), cols 64*(h//2)
            for b in range(2):
                for h in range(HEADS):
                    pb = 64 * (h % 2)
                    cg = 64 * (h // 2)
                    nc.tensor.matmul(
                        s_ps[b][pb : pb + 64, cg : cg + 64],
                        a_sb[b][:, h * 64 : (h + 1) * 64],
                        wk[:, h * 64 : (h + 1) * 64],
                        start=True, stop=True,
                    )
            # Per-group max subtracted on DVE (cheap, parallel engine) so the
            # exp is ONE wide ACT op per batch instead of 8 serialized ones.
            sm_sb = [work.tile([P, 256], F32, tag=f"sm{b}", name=f"sm_sb{b}") for b in range(2)]
            for b in range(2):
                nc.vector.reduce_max(
                    negmax[b][:],
                    s_ps[b][:].rearrange("p (g j) -> p g j", j=64),
                    axis=mybir.AxisListType.X,
                    negate=True,
                )
            for b in range(2):
                nc.vector.tensor_tensor(
                    sm_sb[b][:].rearrange("p (g j) -> p g j", j=64),
                    s_ps[b][:].rearrange("p (g j) -> p g j", j=64),
                    negmax[b][:].rearrange("p g -> p g ()").broadcast_to((P, 4, 64)),
                    op=mybir.AluOpType.add,
                )
            for b in range(2):
                nc.scalar.activation(
                    exp_sb[b][:],
                    sm_sb[b][:],
                    mybir.ActivationFunctionType.Exp,
                    bias=0.0,
                    scale=1.0,
                )
            for b in range(2):
                nc.vector.reduce_sum(
                    sums[:, b * 4 : (b + 1) * 4],
                    exp_sb[b][:].rearrange("p (g j) -> p g j", j=64),
                    axis=mybir.AxisListType.X,
                )
            for b in range(2):
                nc.vector.reciprocal(
                    recip[:, b * 4 : (b + 1) * 4], sums[:, b * 4 : (b + 1) * 4]
                )
            for b in range(2):
                nc.vector.tensor_tensor(
                    attn[b][:].rearrange("p (g j) -> p g j", j=64),
                    exp_sb[b][:].rearrange("p (g j) -> p g j", j=64),
                    recip[:, b * 4 : (b + 1) * 4]
                    .rearrange("p g -> p g ()")
                    .broadcast_to((P, 4, 64)),
                    op=mybir.AluOpType.mult,
                )
            # MT_bh = attn_bh^T @ WoutT_h, same packing as attn/woT
            for b in range(2):
                for h in range(HEADS):
                    pb = 64 * (h % 2)
                    cg = 64 * (h // 2)
                    nc.tensor.matmul(
                        mt_ps[b][pb : pb + 64, cg : cg + 64],
                        attn[b][pb : pb + 64, cg : cg + 64],
                        wo[pb : pb + 64, cg : cg + 64],
                        start=True, stop=True,
                    )
            for b in range(2):
                nc.vector.tensor_copy(mt_sb[b][:], mt_ps[b][:])
            # W_effT_b[c, o] accumulated over the 4 head-pair chunks
            for b in range(2):
                for g in range(4):
                    nc.tensor.matmul(
                        w_ps[b][:],
                        wv[:, g * P : (g + 1) * P],
                        mt_sb[b][:, g * 64 : (g + 1) * 64],
                        start=(g == 0), stop=(g == 3),
                    )
            for b in range(2):
                nc.vector.tensor_copy(weff[b][:], w_ps[b][:])

            # ---- phase E: y = W_eff @ x + b_out, chunked + streamed out ----
            for j in range(8):
                y_ps = psy.tile([P, 512], F32, tag="y", name=f"y_ps{j}")
                for b in range(2):
                    nc.tensor.matmul(
                        y_ps[b * 64 : (b + 1) * 64, :],
                        weff[b][:],
                        xc[:, b * NSH + j * 512 : b * NSH + (j + 1) * 512],
                        start=True, stop=True,
                    )
                y_sb = ypool.tile([P, 512], F32, tag=f"y{j}", name=f"y_sb{j}")
                nc.any.tensor_scalar_add(y_sb[:], y_ps[:], bo[:, 0:1])
                if j < 7:
                    eng = nc.sync if j % 2 == 0 else nc.scalar
                    eng.dma_start(out_ext[:, j * 512 : (j + 1) * 512], y_sb[:])
                else:
                    # split the final chunk across both rings to shorten the
                    # tail (its DMA is the last data movement in the kernel)
                    nc.sync.dma_start(
                        out_ext[:, j * 512 : j * 512 + 256], y_sb[:, 0:256]
                    )
                    nc.scalar.dma_start(
                        out_ext[:, j * 512 + 256 : (j + 1) * 512], y_sb[:, 256:512]
                    )

            # consume the ncfw warm-up collective's output so nothing prunes
            # it; by now its mesh has long completed, so this is free.
            warm_sink = work.tile([P, 2], F32, tag="warm_sink")
            nc.sync.dma_start(warm_sink[:], warm_out[:])

    nc.compile()
    return nc


def _get_nc():
    global _CACHED_NC
    if _CACHED_NC is None:
        _CACHED_NC = build_nc()
    return _CACHED_NC


def make_in_maps(x, w_qkv, w_out, b_out):
    x = np.ascontiguousarray(x, dtype=np.float32)
    w_qkv = np.asarray(w_qkv, dtype=np.float32)
    w_out = np.asarray(w_out, dtype=np.float32)
    b_out = np.asarray(b_out, dtype=np.float32)
    xf = x.reshape(2, P, N_TOT)

    wq_h = np.ascontiguousarray((w_qkv[:512].T * SCALE)).astype(bf16)
    wk_h = np.ascontiguousarray(w_qkv[512:1024].T).astype(bf16)
    wv_h = np.ascontiguousarray(
        (w_qkv[1024:] / N_TOT).reshape(4, P, P).transpose(1, 0, 2).reshape(P, 512)
    ).astype(bf16)
    wo_f = np.zeros((P, 256), np.float32)
    for h in range(HEADS):
        wo_f[
            64 * (h % 2) : 64 * (h % 2) + 64, 64 * (h // 2) : 64 * (h // 2) + 64
        ] = w_out[:, h * 64 : (h + 1) * 64].T
    wo_h = wo_f.astype(bf16)
    bo_h = np.concatenate([b_out, b_out]).reshape(P, 1).astype(np.float32)
    id_h = np.eye(P, dtype=np.float32).astype(bf16)

    in_maps = []
    for c in range(NCORES):
        sh = xf[:, :, c * NSH : (c + 1) * NSH]  # (2, 128, 4096)
        xn_h = np.ascontiguousarray(
            sh.transpose(0, 2, 1)
            .reshape(2, 32, P, P)
            .transpose(2, 0, 1, 3)
            .reshape(P, F)
        ).astype(bf16)
        in_maps.append(
            {
                "xn": xn_h,
                "wqT": wq_h,
                "wkT": wk_h,
                "wv": wv_h,
                "woT": wo_h,
                "bout": bo_h,
                "ident": id_h,
            }
        )
    return in_maps


def assemble_output(results):
    y = np.empty((2, 64, N_TOT), np.float32)
    for c in range(NCORES):
        o = np.asarray(results[c]["out"])  # [128, 4096]
        y[0, :, c * NSH : (c + 1) * NSH] = o[:64]
        y[1, :, c * NSH : (c + 1) * NSH] = o[64:]
    return y.reshape(2, 64, 32, 32, 32)


def kernel(**inputs):
    in_maps = make_in_maps(
        inputs["x"], inputs["w_qkv"], inputs["w_out"], inputs["b_out"]
    )
    nc = _get_nc()
    res = run_bass_kernel_spmd(nc, in_maps, core_ids=list(range(NCORES)))
    return assemble_output(res.results)



# revision 2
# speedup vs baseline: 1.5989x; 1.5989x over previous
"""Trainium2 Bass kernel for nn_Attention (channel-attention, 8 NeuronCores).

Algorithm (algebraically identical to the reference):
  The attention contracts over the spatial axis n = 32*32*32 = 32768, and the
  attention matrices are tiny (64x64 per head).  Everything collapses around
  the per-batch Gram matrix G_b = x_b @ x_b^T (128x128):

    scores_bh = scale * Wq_h G_b Wk_h^T            (tiny)
    attn      = softmax(scores)                     (tiny)
    W_eff_b   = (1/n) * sum_h Wout_h attn_bh Wv_h   (64x128, tiny)
    y_b       = W_eff_b @ x_b + b_out               (the only other big matmul)

  Sharding: NO collectives.  On this stack an ncfw collective costs
  60-80us of firmware-wakeup latency (measured: a self-group warm-up
  AllReduce triggered at t=3us completes at t=61-79us on every core),
  which dwarfs the whole computation.  Instead every core receives the
  FULL x in fp8-e4m3 [n, c] layout (8 MB) and computes the complete Gram
  redundantly; fp8 is harmless here because the Gram contracts over
  32768 samples (measured end-to-end max rel err ~2e-3, dominated by the
  bf16 output, not fp8).  Each core also receives its own 1/8 spatial
  output shard in bf16 [c, n] layout (2 MB) for the y matmul, and writes
  its y shard in bf16.  Per-core cost is DMA-bound: ~10.5 MB of input at
  ~358 GB/s.

  The Gram runs as fp8 DoubleRow matmuls (2 contraction rows/cycle).
  Batch 0's Gram only needs the first half of the stream, so batch 0's
  attention algebra, y matmul and output DMA all hide under batch 1's
  input stream.
"""

import numpy as np
import ml_dtypes

import concourse.bass as bass
import concourse.bacc as bacc
import concourse.mybir as mybir
import concourse.tile as tile
from concourse.bass_utils import run_bass_kernel_spmd

NCORES = 8
P = 128
N_TOT = 32 * 32 * 32          # 32768 spatial points
NSH = N_TOT // NCORES         # 4096 per core per batch (output shard)
SUB = N_TOT // P              # 256 fp8 k-subtiles per batch
CHUNK_SUB = 16                # subtiles per DMA chunk
NCHUNK = SUB // CHUNK_SUB     # 16 chunks per batch (256 KB each)
CHW = CHUNK_SUB * P           # 2048 fp8 free columns per chunk
HEADS = 8
DH = 64
SCALE = DH ** -0.5
BF = mybir.dt.bfloat16
F32 = mybir.dt.float32
FP8 = mybir.dt.float8e4
DR = mybir.MatmulPerfMode.DoubleRow
bf16 = ml_dtypes.bfloat16
f8 = ml_dtypes.float8_e4m3

_CACHED_NC = None


class _TrimmedTileContext(tile.TileContext):
    """TileContext minus the FINAL all-engine barrier of the exit sequence.

    The stock exit is drain -> barrier -> sem-clear -> barrier; the last
    barrier only makes every engine wait for the gpsimd sem-clear before
    halting, which matters for looped NEFFs but not a single-shot kernel:
    the clear still completes before its issuing engine halts, so a
    re-execution starts with zeroed semaphores either way.  Dropping it
    saves ~4us of measured EVSEM-butterfly tail.
    """

    def _drain_and_barrier(self, tick_clock, wait_clock):
        from concourse.vector_clock import ScopedClock

        drain_inst = self.nc.sync.drain()
        wait_clock.add_sem_waits(
            drain_inst.ins, ScopedClock({None: tick_clock.global_clock})
        )
        self.nc.all_engine_barrier()
        popped = self.nc._tile_sem_poison_stack.pop()
        assert popped is self._sem_poison
        self.nc.clear_and_free_semaphores(list(self.sems.allocated().values()))


def build_nc():
    nc = bacc.Bacc(
        "TRN2", target_bir_lowering=False, debug=False, num_devices=NCORES
    )

    # full x, fp8, [p, (b, m, c)] DoubleRow layout: subtile m holds spatial
    # rows m*128..m*128+127 of batch b, channels on the innermost axis.
    xg_ext = nc.dram_tensor("xg", [P, 2 * SUB * P], FP8, kind="ExternalInput")
    # own output shard, bf16, [c, (b, n)] layout for the y matmul
    xc_ext = nc.dram_tensor("xc", [P, 2 * NSH], BF, kind="ExternalInput")
    wq_ext = nc.dram_tensor("wqT", [P, 512], BF, kind="ExternalInput")
    wk_ext = nc.dram_tensor("wkT", [P, 512], BF, kind="ExternalInput")
    wv_ext = nc.dram_tensor("wv", [P, 512], BF, kind="ExternalInput")
    wo_ext = nc.dram_tensor("woT", [P, 256], BF, kind="ExternalInput")
    bo_ext = nc.dram_tensor("bout", [P, 1], F32, kind="ExternalInput")
    out_ext = nc.dram_tensor("out", [P, NSH], BF, kind="ExternalOutput")

    with _TrimmedTileContext(nc) as tc:
        with (
            tc.tile_pool(name="const", bufs=1) as const,
            tc.tile_pool(name="data", bufs=1) as data,
            tc.tile_pool(name="work", bufs=1) as work,
            tc.tile_pool(name="ypool", bufs=2) as ypool,
            tc.tile_pool(name="psg", bufs=2, space="PSUM") as psg,
            tc.tile_pool(name="psd", bufs=2, space="PSUM") as psd,
            tc.tile_pool(name="psy", bufs=2, space="PSUM") as psy,
        ):
            # ---- input DMAs, program order == ring FIFO order ----
            # batch-0 Gram stream first, then weights (needed at ~12.5us for
            # phase D), then xc batch 0 (needed for phase E0), then the
            # batch-1 Gram stream, then xc batch 1.
            xg_tiles = [[], []]
            xc = data.tile([P, 2 * NSH], BF, tag="xc")

            def queue_xg(b):
                for c in range(NCHUNK):
                    t = data.tile([P, CHW], FP8, tag=f"xg{b}_{c}")
                    eng = nc.sync if c % 2 == 0 else nc.scalar
                    off = (b * SUB + c * CHUNK_SUB) * P
                    eng.dma_start(t[:], xg_ext[:, off : off + CHW])
                    xg_tiles[b].append(t)

            queue_xg(0)

            wq = const.tile([P, 512], BF, tag="wq")
            nc.sync.dma_start(wq[:], wq_ext[:])
            wk = const.tile([P, 512], BF, tag="wk")
            nc.scalar.dma_start(wk[:], wk_ext[:])
            wv = const.tile([P, 512], BF, tag="wv")
            nc.sync.dma_start(wv[:], wv_ext[:])
            wo = const.tile([P, 256], BF, tag="wo")
            nc.scalar.dma_start(wo[:], wo_ext[:])
            bo = const.tile([P, 1], F32, tag="bo")
            nc.sync.dma_start(bo[:], bo_ext[:])

            # xc batch 0 split across both rings
            nc.sync.dma_start(xc[:, 0 : NSH // 2], xc_ext[:, 0 : NSH // 2])
            nc.scalar.dma_start(xc[:, NSH // 2 : NSH], xc_ext[:, NSH // 2 : NSH])

            queue_xg(1)

            nc.sync.dma_start(
                xc[:, NSH : NSH + NSH // 2], xc_ext[:, NSH : NSH + NSH // 2]
            )
            nc.scalar.dma_start(
                xc[:, NSH + NSH // 2 :], xc_ext[:, NSH + NSH // 2 :]
            )

            # ---- per-batch pipeline: Gram -> phase D -> phase E ----
            g_ps = [None, None]
            gbf = [None, None]

            def gram(b):
                g_ps[b] = psg.tile([P, P], F32, tag="g", name=f"g_ps{b}")
                n_mm = CHUNK_SUB // 2
                for c, t in enumerate(xg_tiles[b]):
                    xr = t[:].rearrange("p (m c) -> p m c", c=P)
                    for j in range(n_mm):
                        sl = xr[:, 2 * j : 2 * j + 2, :]
                        nc.tensor.matmul(
                            g_ps[b][:], sl, sl,
                            start=(c == 0 and j == 0),
                            stop=(c == NCHUNK - 1 and j == n_mm - 1),
                            perf_mode=DR,
                        )
                gbf[b] = work.tile([P, P], BF, tag=f"gbf{b}", name=f"gbf{b}")
                nc.vector.tensor_copy(gbf[b][:], g_ps[b][:])

            def phase_d(b):
                """scores -> softmax -> W_eff for batch b (all tiny)."""
                a_ps = psd.tile([P, 512], F32, tag="d", name=f"a_ps{b}")
                a_sb = work.tile([P, 512], BF, tag=f"asb{b}", name=f"a_sb{b}")
                s_ps = psd.tile([P, 256], F32, tag="d", name=f"s_ps{b}")
                negmax = work.tile([P, 4], F32, tag=f"nm{b}", name=f"negmax{b}")
                sm_sb = work.tile([P, 256], F32, tag=f"sm{b}", name=f"sm_sb{b}")
                exp_sb = work.tile([P, 256], F32, tag=f"exp{b}", name=f"exp_sb{b}")
                sums = work.tile([P, 4], F32, tag=f"sums{b}", name=f"sums{b}")
                recip = work.tile([P, 4], F32, tag=f"recip{b}", name=f"recip{b}")
                attn = work.tile([P, 256], BF, tag=f"attn{b}", name=f"attn{b}")
                mt_ps = psd.tile([P, 256], F32, tag="d2", name=f"mt_ps{b}")
                mt_sb = work.tile([P, 256], BF, tag=f"mt{b}", name=f"mt_sb{b}")
                w_ps = psd.tile([P, 64], F32, tag="d2", name=f"w_ps{b}")
                weff = work.tile([P, 64], BF, tag=f"weff{b}", name=f"weff{b}")

                nc.tensor.matmul(
                    a_ps[:], gbf[b][:], wq[:], start=True, stop=True
                )
                # sliced so the first S matmuls start after slice 0 lands
                for sl in range(4):
                    nc.vector.tensor_copy(
                        a_sb[:, sl * 128 : (sl + 1) * 128],
                        a_ps[:, sl * 128 : (sl + 1) * 128],
                    )
                # S[i-half, j-group]: head h at partitions 64*(h%2),
                # cols 64*(h//2)
                for h in range(HEADS):
                    pb = 64 * (h % 2)
                    cg = 64 * (h // 2)
                    nc.tensor.matmul(
                        s_ps[pb : pb + 64, cg : cg + 64],
                        a_sb[:, h * 64 : (h + 1) * 64],
                        wk[:, h * 64 : (h + 1) * 64],
                        start=True, stop=True,
                    )
                # per-group max on DVE so the exp is ONE wide ACT op
                nc.vector.reduce_max(
                    negmax[:],
                    s_ps[:].rearrange("p (g j) -> p g j", j=64),
                    axis=mybir.AxisListType.X,
                    negate=True,
                )
                nc.vector.tensor_tensor(
                    sm_sb[:].rearrange("p (g j) -> p g j", j=64),
                    s_ps[:].rearrange("p (g j) -> p g j", j=64),
                    negmax[:].rearrange("p g -> p g ()").broadcast_to((P, 4, 64)),
                    op=mybir.AluOpType.add,
                )
                nc.scalar.activation(
                    exp_sb[:],
                    sm_sb[:],
                    mybir.ActivationFunctionType.Exp,
                    bias=0.0,
                    scale=1.0,
                )
                nc.vector.reduce_sum(
                    sums[:],
                    exp_sb[:].rearrange("p (g j) -> p g j", j=64),
                    axis=mybir.AxisListType.X,
                )
                nc.vector.reciprocal(recip[:], sums[:])
                nc.vector.tensor_tensor(
                    attn[:].rearrange("p (g j) -> p g j", j=64),
                    exp_sb[:].rearrange("p (g j) -> p g j", j=64),
                    recip[:].rearrange("p g -> p g ()").broadcast_to((P, 4, 64)),
                    op=mybir.AluOpType.mult,
                )
                # MT_bh = attn_bh^T @ WoutT_h, same packing as attn/woT
                for h in range(HEADS):
                    pb = 64 * (h % 2)
                    cg = 64 * (h // 2)
                    nc.tensor.matmul(
                        mt_ps[pb : pb + 64, cg : cg + 64],
                        attn[pb : pb + 64, cg : cg + 64],
                        wo[pb : pb + 64, cg : cg + 64],
                        start=True, stop=True,
                    )
                nc.vector.tensor_copy(mt_sb[:], mt_ps[:])
                # W_effT_b[c, o] accumulated over the 4 head-pair chunks
                for g in range(4):
                    nc.tensor.matmul(
                        w_ps[:],
                        wv[:, g * P : (g + 1) * P],
                        mt_sb[:, g * 64 : (g + 1) * 64],
                        start=(g == 0), stop=(g == 3),
                    )
                nc.vector.tensor_copy(weff[:], w_ps[:])
                return weff

            def phase_e(b, weff):
                """y_b = W_eff_b @ x_b + b_out, chunked + streamed out."""
                for j in range(8):
                    y_ps = psy.tile([64, 512], F32, tag="y", name=f"y_ps{b}_{j}")
                    nc.tensor.matmul(
                        y_ps[:],
                        weff[:],
                        xc[:, b * NSH + j * 512 : b * NSH + (j + 1) * 512],
                        start=True, stop=True,
                    )
                    y_sb = ypool.tile([64, 512], BF, tag="ysb", name=f"y_sb{b}_{j}")
                    nc.any.tensor_scalar_add(y_sb[:], y_ps[:], bo[0:64, 0:1])
                    eng = nc.sync if j % 2 == 0 else nc.scalar
                    eng.dma_start(
                        out_ext[b * 64 : (b + 1) * 64, j * 512 : (j + 1) * 512],
                        y_sb[:],
                    )

            for b in range(2):
                gram(b)
                weff = phase_d(b)
                phase_e(b, weff)

    nc.compile()
    return nc


def _get_nc():
    global _CACHED_NC
    if _CACHED_NC is None:
        _CACHED_NC = build_nc()
    return _CACHED_NC


def make_in_maps(x, w_qkv, w_out, b_out):
    x = np.ascontiguousarray(x, dtype=np.float32)
    w_qkv = np.asarray(w_qkv, dtype=np.float32)
    w_out = np.asarray(w_out, dtype=np.float32)
    b_out = np.asarray(b_out, dtype=np.float32)
    xf = x.reshape(2, P, N_TOT)

    # full x, fp8, DoubleRow layout [p, b, m, c]
    xg_h = np.ascontiguousarray(
        xf.transpose(0, 2, 1)            # (2, n, c)
        .reshape(2, SUB, P, P)           # (2, m, p, c)
        .transpose(2, 0, 1, 3)           # (p, 2, m, c)
        .reshape(P, 2 * SUB * P)
    ).astype(f8)

    wq_h = np.ascontiguousarray((w_qkv[:512].T * SCALE)).astype(bf16)
    wk_h = np.ascontiguousarray(w_qkv[512:1024].T).astype(bf16)
    wv_h = np.ascontiguousarray(
        (w_qkv[1024:] / N_TOT).reshape(4, P, P).transpose(1, 0, 2).reshape(P, 512)
    ).astype(bf16)
    wo_f = np.zeros((P, 256), np.float32)
    for h in range(HEADS):
        wo_f[
            64 * (h % 2) : 64 * (h % 2) + 64, 64 * (h // 2) : 64 * (h // 2) + 64
        ] = w_out[:, h * 64 : (h + 1) * 64].T
    wo_h = wo_f.astype(bf16)
    bo_h = np.concatenate([b_out, b_out]).reshape(P, 1).astype(np.float32)

    in_maps = []
    for c in range(NCORES):
        # own output shard, bf16, [c, (b, n)]
        xc_h = np.ascontiguousarray(
            xf[:, :, c * NSH : (c + 1) * NSH].transpose(1, 0, 2).reshape(P, 2 * NSH)
        ).astype(bf16)
        in_maps.append(
            {
                "xg": xg_h,
                "xc": xc_h,
                "wqT": wq_h,
                "wkT": wk_h,
                "wv": wv_h,
                "woT": wo_h,
                "bout": bo_h,
            }
        )
    return in_maps


def assemble_output(results):
    y = np.empty((2, 64, N_TOT), np.float32)
    for c in range(NCORES):
        o = np.asarray(results[c]["out"]).astype(np.float32)  # [128, 4096] bf16
        y[0, :, c * NSH : (c + 1) * NSH] = o[:64]
        y[1, :, c * NSH : (c + 1) * NSH] = o[64:]
    return y.reshape(2, 64, 32, 32, 32)


def kernel(**inputs):
    in_maps = make_in_maps(
        inputs["x"], inputs["w_qkv"], inputs["w_out"], inputs["b_out"]
    )
    nc = _get_nc()
    res = run_bass_kernel_spmd(nc, in_maps, core_ids=list(range(NCORES)))
    return assemble_output(res.results)


# revision 10
# speedup vs baseline: 1.6532x; 1.0339x over previous
"""Trainium2 Bass kernel for nn_Attention (channel-attention, 8 NeuronCores).

Algorithm (algebraically identical to the reference):
  The attention contracts over the spatial axis n = 32*32*32 = 32768, and the
  attention matrices are tiny (64x64 per head).  Everything collapses around
  the per-batch Gram matrix G_b = x_b @ x_b^T (128x128):

    scores_bh = scale * Wq_h G_b Wk_h^T            (tiny)
    attn      = softmax(scores)                     (tiny)
    W_eff_b   = (1/n) * sum_h Wout_h attn_bh Wv_h   (64x128, tiny)
    y_b       = W_eff_b @ x_b + b_out               (the only other big matmul)

  Sharding: NO collectives.  On this stack an ncfw collective costs
  60-80us of firmware-wakeup latency (measured: a self-group warm-up
  AllReduce triggered at t=3us completes at t=61-79us on every core),
  which dwarfs the whole computation.  Instead every core receives the
  FULL x in fp8-e4m3 [n, c] layout (8 MB) and computes the complete Gram
  redundantly; fp8 is harmless here because the Gram contracts over
  32768 samples (measured end-to-end max rel err ~2e-3, dominated by the
  bf16 output, not fp8).  Each core also receives its own 1/8 spatial
  output shard in bf16 [c, n] layout (2 MB) for the y matmul, and writes
  its y shard in bf16.  Per-core cost is DMA-bound: ~10.5 MB of input at
  ~350 GB/s.

  The Gram runs as fp8 DoubleRow matmuls (2 contraction rows/cycle).
  Stream order: weights+xc(batch0) first, then the batch-0 Gram stream,
  then batch 1's, then xc(batch1) -- so batch 0's attention algebra and
  its entire y half (matmuls, bias, output DMA) hide under batch 1's
  input stream, and only batch 1's tiny tail runs after the last byte.
"""

import numpy as np
import ml_dtypes

import concourse.bass as bass
import concourse.bacc as bacc
import concourse.mybir as mybir
import concourse.tile as tile
from concourse.bass_utils import run_bass_kernel_spmd

NCORES = 8
P = 128
N_TOT = 32 * 32 * 32          # 32768 spatial points
NSH = N_TOT // NCORES         # 4096 per core per batch (output shard)
SUB = N_TOT // P              # 256 fp8 k-subtiles per batch
CHUNK_SUB = 32                # subtiles per DMA chunk (512 KB)
NCHUNK = SUB // CHUNK_SUB     # 8 chunks per batch
CHW = CHUNK_SUB * P           # 4096 fp8 free columns per chunk
HEADS = 8
DH = 64
SCALE = DH ** -0.5
WCOLS = 512 + 512 + 512 + 256 + 1  # packed weights: wq|wk|wv|wo|bo
BF = mybir.dt.bfloat16
F32 = mybir.dt.float32
FP8 = mybir.dt.float8e4
DR = mybir.MatmulPerfMode.DoubleRow
bf16 = ml_dtypes.bfloat16
f8 = ml_dtypes.float8_e4m3

_CACHED_NC = None


class _TrimmedTileContext(tile.TileContext):
    """TileContext minus the FINAL all-engine barrier of the exit sequence.

    The stock exit is drain -> barrier -> sem-clear -> barrier; the last
    barrier only makes every engine wait for the gpsimd sem-clear before
    halting, which matters for looped NEFFs but not a single-shot kernel:
    the clear still completes before its issuing engine halts, so a
    re-execution starts with zeroed semaphores either way.  Dropping it
    saves ~4us of measured EVSEM-butterfly tail.
    """

    def _drain_and_barrier(self, tick_clock, wait_clock):
        from concourse.vector_clock import ScopedClock

        drain_inst = self.nc.sync.drain()
        wait_clock.add_sem_waits(
            drain_inst.ins, ScopedClock({None: tick_clock.global_clock})
        )
        self.nc.all_engine_barrier()
        popped = self.nc._tile_sem_poison_stack.pop()
        assert popped is self._sem_poison
        self.nc.clear_and_free_semaphores(list(self.sems.allocated().values()))


def build_nc():
    nc = bacc.Bacc(
        "TRN2", target_bir_lowering=False, debug=False, num_devices=NCORES
    )

    # full x, fp8, [p, (b, m, c)] DoubleRow layout: subtile m holds spatial
    # rows m*128..m*128+127 of batch b, channels on the innermost axis.
    xg_ext = nc.dram_tensor("xg", [P, 2 * SUB * P], FP8, kind="ExternalInput")
    # own output shard, bf16, [c, (b, n)] layout for the y matmul
    xc_ext = nc.dram_tensor("xc", [P, 2 * NSH], BF, kind="ExternalInput")
    w_ext = nc.dram_tensor("wpack", [P, WCOLS], BF, kind="ExternalInput")
    out_ext = nc.dram_tensor("out", [P, NSH], BF, kind="ExternalOutput")

    with _TrimmedTileContext(nc) as tc:
        with (
            tc.tile_pool(name="const", bufs=1) as const,
            tc.tile_pool(name="data", bufs=1) as data,
            tc.tile_pool(name="work", bufs=1) as work,
            tc.tile_pool(name="ypool", bufs=2) as ypool,
            tc.tile_pool(name="psg", bufs=1, space="PSUM") as psg,
            tc.tile_pool(name="psd", bufs=2, space="PSUM") as psd,
            tc.tile_pool(name="psy", bufs=2, space="PSUM") as psy,
        ):
            # ---- input DMAs, program order == ring FIFO order ----
            # weights + xc(batch 0) FIRST so phase D0/E0 never stall the
            # PE queue mid-stream, then the two Gram streams, then
            # xc(batch 1) (only needed by the final tail).
            wpack = const.tile([P, WCOLS], BF, tag="wpack")
            nc.sync.dma_start(wpack[:], w_ext[:])
            wq = wpack[:, 0:512]
            wk = wpack[:, 512:1024]
            wv = wpack[:, 1024:1536]
            wo = wpack[:, 1536:1792]
            # tensor_scalar wants an f32 scalar operand; upcast bias once
            bo = work.tile([P, 1], F32, tag="bo")
            nc.vector.tensor_copy(bo[:], wpack[:, 1792:1793])

            xc = data.tile([P, 2 * NSH], BF, tag="xc")
            nc.scalar.dma_start(xc[:, 0:NSH], xc_ext[:, 0:NSH])

            xg_tiles = [[], []]

            def queue_xg(b):
                for c in range(NCHUNK):
                    t = data.tile([P, CHW], FP8, tag=f"xg{b}_{c}")
                    eng = nc.sync if c % 2 == 0 else nc.scalar
                    off = (b * SUB + c * CHUNK_SUB) * P
                    eng.dma_start(t[:], xg_ext[:, off : off + CHW])
                    xg_tiles[b].append(t)

            queue_xg(0)
            queue_xg(1)

            nc.sync.dma_start(
                xc[:, NSH : NSH + NSH // 2], xc_ext[:, NSH : NSH + NSH // 2]
            )
            nc.scalar.dma_start(
                xc[:, NSH + NSH // 2 :], xc_ext[:, NSH + NSH // 2 :]
            )

            # ---- per-batch pipeline: Gram -> phase D -> phase E ----
            g_ps = [None, None]
            gbf = [None, None]

            def gram(b):
                g_ps[b] = psg.tile([P, P], F32, tag="g", name=f"g_ps{b}")
                n_mm = CHUNK_SUB // 2
                for c, t in enumerate(xg_tiles[b]):
                    xr = t[:].rearrange("p (m c) -> p m c", c=P)
                    for j in range(n_mm):
                        sl = xr[:, 2 * j : 2 * j + 2, :]
                        nc.tensor.matmul(
                            g_ps[b][:], sl, sl,
                            start=(c == 0 and j == 0),
                            stop=(c == NCHUNK - 1 and j == n_mm - 1),
                            perf_mode=DR,
                        )
                gbf[b] = work.tile([P, P], BF, tag=f"gbf{b}", name=f"gbf{b}")
                nc.vector.tensor_copy(gbf[b][:], g_ps[b][:])

            def phase_d(b):
                """scores -> softmax -> W_eff for batch b (all tiny)."""
                a_ps = psd.tile([P, 512], F32, tag="d", name=f"a_ps{b}")
                a_sb = work.tile([P, 512], BF, tag=f"asb{b}", name=f"a_sb{b}")
                s_ps = psd.tile([P, 256], F32, tag="d", name=f"s_ps{b}")
                negmax = work.tile([P, 4], F32, tag=f"nm{b}", name=f"negmax{b}")
                sm_sb = work.tile([P, 256], F32, tag=f"sm{b}", name=f"sm_sb{b}")
                exp_sb = work.tile([P, 256], F32, tag=f"exp{b}", name=f"exp_sb{b}")
                sums = work.tile([P, 4], F32, tag=f"sums{b}", name=f"sums{b}")
                recip = work.tile([P, 4], F32, tag=f"recip{b}", name=f"recip{b}")
                attn = work.tile([P, 256], BF, tag=f"attn{b}", name=f"attn{b}")
                mt_ps = psd.tile([P, 256], F32, tag="d", name=f"mt_ps{b}")
                mt_sb = work.tile([P, 256], BF, tag=f"mt{b}", name=f"mt_sb{b}")
                w_ps = psd.tile([P, 64], F32, tag="d", name=f"w_ps{b}")
                weff = work.tile([P, 64], BF, tag=f"weff{b}", name=f"weff{b}")

                nc.tensor.matmul(
                    a_ps[:], gbf[b][:], wq, start=True, stop=True
                )
                # sliced so the first S matmuls start after slice 0 lands
                for sl in range(4):
                    nc.vector.tensor_copy(
                        a_sb[:, sl * 128 : (sl + 1) * 128],
                        a_ps[:, sl * 128 : (sl + 1) * 128],
                    )
                # S[i-half, j-group]: head h at partitions 64*(h%2),
                # cols 64*(h//2)
                for h in range(HEADS):
                    pb = 64 * (h % 2)
                    cg = 64 * (h // 2)
                    nc.tensor.matmul(
                        s_ps[pb : pb + 64, cg : cg + 64],
                        a_sb[:, h * 64 : (h + 1) * 64],
                        wk[:, h * 64 : (h + 1) * 64],
                        start=True, stop=True,
                    )
                # per-group max on DVE so the exp is ONE wide ACT op
                nc.vector.reduce_max(
                    negmax[:],
                    s_ps[:].rearrange("p (g j) -> p g j", j=64),
                    axis=mybir.AxisListType.X,
                    negate=True,
                )
                nc.vector.tensor_tensor(
                    sm_sb[:].rearrange("p (g j) -> p g j", j=64),
                    s_ps[:].rearrange("p (g j) -> p g j", j=64),
                    negmax[:].rearrange("p g -> p g ()").broadcast_to((P, 4, 64)),
                    op=mybir.AluOpType.add,
                )
                nc.scalar.activation(
                    exp_sb[:],
                    sm_sb[:],
                    mybir.ActivationFunctionType.Exp,
                    bias=0.0,
                    scale=1.0,
                )
                nc.vector.reduce_sum(
                    sums[:],
                    exp_sb[:].rearrange("p (g j) -> p g j", j=64),
                    axis=mybir.AxisListType.X,
                )
                nc.vector.reciprocal(recip[:], sums[:])
                nc.vector.tensor_tensor(
                    attn[:].rearrange("p (g j) -> p g j", j=64),
                    exp_sb[:].rearrange("p (g j) -> p g j", j=64),
                    recip[:].rearrange("p g -> p g ()").broadcast_to((P, 4, 64)),
                    op=mybir.AluOpType.mult,
                )
                # MT_bh = attn_bh^T @ WoutT_h, same packing as attn/woT
                for h in range(HEADS):
                    pb = 64 * (h % 2)
                    cg = 64 * (h // 2)
                    nc.tensor.matmul(
                        mt_ps[pb : pb + 64, cg : cg + 64],
                        attn[pb : pb + 64, cg : cg + 64],
                        wo[pb : pb + 64, cg : cg + 64],
                        start=True, stop=True,
                    )
                nc.vector.tensor_copy(mt_sb[:], mt_ps[:])
                # W_effT_b[c, o] accumulated over the 4 head-pair chunks
                for g in range(4):
                    nc.tensor.matmul(
                        w_ps[:],
                        wv[:, g * P : (g + 1) * P],
                        mt_sb[:, g * 64 : (g + 1) * 64],
                        start=(g == 0), stop=(g == 3),
                    )
                nc.vector.tensor_copy(weff[:], w_ps[:])
                return weff

            def phase_e(b, weff):
                """y_b = W_eff_b @ x_b + b_out, 4 wide chunks, streamed out."""
                for j in range(4):
                    y_ps = psy.tile([64, 1024], F32, tag="y", name=f"y_ps{b}_{j}")
                    for h in range(2):  # one matmul per PSUM bank
                        nc.tensor.matmul(
                            y_ps[:, h * 512 : (h + 1) * 512],
                            weff[:],
                            xc[
                                :,
                                b * NSH + j * 1024 + h * 512 :
                                b * NSH + j * 1024 + (h + 1) * 512,
                            ],
                            start=True, stop=True,
                        )
                    y_sb = ypool.tile(
                        [64, 1024], BF, tag="ysb", name=f"y_sb{b}_{j}"
                    )
                    nc.any.tensor_scalar_add(y_sb[:], y_ps[:], bo[0:64, 0:1])
                    deng = nc.sync if j % 2 == 0 else nc.scalar
                    deng.dma_start(
                        out_ext[b * 64 : (b + 1) * 64, j * 1024 : (j + 1) * 1024],
                        y_sb[:],
                    )

            for b in range(2):
                gram(b)
                weff = phase_d(b)
                phase_e(b, weff)

    nc.compile()
    return nc


def _get_nc():
    global _CACHED_NC
    if _CACHED_NC is None:
        _CACHED_NC = build_nc()
    return _CACHED_NC


def make_in_maps(x, w_qkv, w_out, b_out):
    x = np.ascontiguousarray(x, dtype=np.float32)
    w_qkv = np.asarray(w_qkv, dtype=np.float32)
    w_out = np.asarray(w_out, dtype=np.float32)
    b_out = np.asarray(b_out, dtype=np.float32)
    xf = x.reshape(2, P, N_TOT)

    # full x, fp8, DoubleRow layout [p, b, m, c]
    xg_h = np.ascontiguousarray(
        xf.transpose(0, 2, 1)            # (2, n, c)
        .reshape(2, SUB, P, P)           # (2, m, p, c)
        .transpose(2, 0, 1, 3)           # (p, 2, m, c)
        .reshape(P, 2 * SUB * P)
    ).astype(f8)

    wpack = np.zeros((P, WCOLS), np.float32)
    wpack[:, 0:512] = w_qkv[:512].T * SCALE
    wpack[:, 512:1024] = w_qkv[512:1024].T
    wpack[:, 1024:1536] = (
        (w_qkv[1024:] / N_TOT).reshape(4, P, P).transpose(1, 0, 2).reshape(P, 512)
    )
    for h in range(HEADS):
        wpack[
            64 * (h % 2) : 64 * (h % 2) + 64,
            1536 + 64 * (h // 2) : 1536 + 64 * (h // 2) + 64,
        ] = w_out[:, h * 64 : (h + 1) * 64].T
    wpack[:, 1792] = np.concatenate([b_out, b_out])
    wpack_h = wpack.astype(bf16)

    in_maps = []
    for c in range(NCORES):
        # own output shard, bf16, [c, (b, n)]
        xc_h = np.ascontiguousarray(
            xf[:, :, c * NSH : (c + 1) * NSH].transpose(1, 0, 2).reshape(P, 2 * NSH)
        ).astype(bf16)
        in_maps.append({"xg": xg_h, "xc": xc_h, "wpack": wpack_h})
    return in_maps


def assemble_output(results):
    y = np.empty((2, 64, N_TOT), np.float32)
    for c in range(NCORES):
        o = np.asarray(results[c]["out"]).astype(np.float32)  # [128, 4096] bf16
        y[0, :, c * NSH : (c + 1) * NSH] = o[:64]
        y[1, :, c * NSH : (c + 1) * NSH] = o[64:]
    return y.reshape(2, 64, 32, 32, 32)


def kernel(**inputs):
    in_maps = make_in_maps(
        inputs["x"], inputs["w_qkv"], inputs["w_out"], inputs["b_out"]
    )
    nc = _get_nc()
    res = run_bass_kernel_spmd(nc, in_maps, core_ids=list(range(NCORES)))
    return assemble_output(res.results)


# revision 13
# speedup vs baseline: 2.0022x; 1.2111x over previous
"""Trainium2 Bass kernel for nn_Attention (channel-attention, 8 NeuronCores).

Algorithm (algebraically identical to the reference):
  The attention contracts over the spatial axis n = 32*32*32 = 32768, and the
  attention matrices are tiny (64x64 per head).  Everything collapses around
  the per-batch Gram matrix G_b = x_b @ x_b^T (128x128):

    scores_bh = scale * Wq_h G_b Wk_h^T            (tiny)
    attn      = softmax(scores)                     (tiny)
    W_eff_b   = (1/n) * sum_h Wout_h attn_bh Wv_h   (64x128, tiny)
    y_b       = W_eff_b @ x_b + b_out               (the only other big matmul)

  Sharding: NO collectives.  On this stack an ncfw collective costs
  60-80us of firmware-wakeup latency (measured: a self-group warm-up
  AllReduce triggered at t=3us completes at t=61-79us on every core),
  which dwarfs the whole computation.  Instead every core receives the
  FULL x in fp8-e4m3 [n, c] layout (8 MB) and computes the complete Gram
  redundantly; fp8 is harmless here because the Gram contracts over
  32768 samples (measured end-to-end max rel err ~2.8e-3, dominated by
  the bf16 output, not fp8).  Each core also receives its own 1/8
  spatial output shard in bf16 [c, n] layout (2 MB) for the y matmul,
  and writes its y shard in bf16.

  Balance: the fp8 stream moves at ~405 GB/s (26us); the Gram runs as
  fp8 DoubleRow matmuls at a measured 93 ns per 256-row pair (24us), so
  the middle section is jointly DMA/PE-bound.  Batch 0's attention
  algebra is interleaved into batch 1's Gram instruction stream at
  points where its DVE/ACT dependencies are already settled, so the PE
  queue never head-of-line blocks; batch 0's output DMAs ride the
  gpsimd SWDGE ring because the two HWDGE rings are busy with the input
  stream FIFO.  A short burst of throwaway fp32 matmuls at the top
  keeps the PE HAM clock-gate warm through the DMA-prefill window.
"""

import numpy as np
import ml_dtypes

import concourse.bass as bass
import concourse.bacc as bacc
import concourse.mybir as mybir
import concourse.tile as tile
from concourse.bass_utils import run_bass_kernel_spmd

NCORES = 8
P = 128
N_TOT = 32 * 32 * 32          # 32768 spatial points
NSH = N_TOT // NCORES         # 4096 per core per batch (output shard)
SUB = N_TOT // P              # 256 fp8 k-subtiles per batch
CHUNK_SUB = 32                # subtiles per DMA chunk (512 KB)
NCHUNK = SUB // CHUNK_SUB     # 8 chunks per batch
CHW = CHUNK_SUB * P           # 4096 fp8 free columns per chunk
HEADS = 8
DH = 64
SCALE = DH ** -0.5
WCOLS = 512 + 512 + 512 + 256 + 1  # packed weights: wq|wk|wv|wo|bo
WARM_MMS = 10                 # fp32 HAM warm-keepers during DMA prefill
BF = mybir.dt.bfloat16
F32 = mybir.dt.float32
FP8 = mybir.dt.float8e4
DR = mybir.MatmulPerfMode.DoubleRow
bf16 = ml_dtypes.bfloat16
f8 = ml_dtypes.float8_e4m3

_CACHED_NC = None


class _TrimmedTileContext(tile.TileContext):
    """TileContext minus the FINAL all-engine barrier of the exit sequence.

    The stock exit is drain -> barrier -> sem-clear -> barrier; the last
    barrier only makes every engine wait for the gpsimd sem-clear before
    halting, which matters for looped NEFFs but not a single-shot kernel:
    the clear still completes before its issuing engine halts, so a
    re-execution starts with zeroed semaphores either way.  Dropping it
    saves ~4us of measured EVSEM-butterfly tail.
    """

    def _drain_and_barrier(self, tick_clock, wait_clock):
        from concourse.vector_clock import ScopedClock

        drain_inst = self.nc.sync.drain()
        wait_clock.add_sem_waits(
            drain_inst.ins, ScopedClock({None: tick_clock.global_clock})
        )
        self.nc.all_engine_barrier()
        popped = self.nc._tile_sem_poison_stack.pop()
        assert popped is self._sem_poison
        self.nc.clear_and_free_semaphores(list(self.sems.allocated().values()))


def build_nc():
    # The stock Bass init ends with const-AP memsets guarded by a second
    # all-engine barrier; the consts are unused here and the barrier adds
    # ~2us of start-up serialization, so skip that one barrier only.
    orig_barrier = bass.Bass.all_engine_barrier
    bass.Bass.all_engine_barrier = lambda self: None
    try:
        nc = bacc.Bacc(
            "TRN2", target_bir_lowering=False, debug=False, num_devices=NCORES
        )
    finally:
        bass.Bass.all_engine_barrier = orig_barrier

    # full x, fp8, [p, (b, m, c)] DoubleRow layout: subtile m holds spatial
    # rows m*128..m*128+127 of batch b, channels on the innermost axis.
    xg_ext = nc.dram_tensor("xg", [P, 2 * SUB * P], FP8, kind="ExternalInput")
    # own output shard, bf16, [c, (b, n)] layout for the y matmul
    xc_ext = nc.dram_tensor("xc", [P, 2 * NSH], BF, kind="ExternalInput")
    w_ext = nc.dram_tensor("wpack", [P, WCOLS], BF, kind="ExternalInput")
    out_ext = nc.dram_tensor("out", [P, NSH], BF, kind="ExternalOutput")

    with _TrimmedTileContext(nc) as tc:
        with (
            tc.tile_pool(name="const", bufs=1) as const,
            tc.tile_pool(name="data", bufs=1) as data,
            tc.tile_pool(name="work", bufs=1) as work,
            tc.tile_pool(name="ypool", bufs=8) as ypool,
            tc.tile_pool(name="psg", bufs=1, space="PSUM") as psg,
            tc.tile_pool(name="psd", bufs=2, space="PSUM") as psd,
            tc.tile_pool(name="psy", bufs=4, space="PSUM") as psy,
            tc.tile_pool(name="psw", bufs=1, space="PSUM") as psw,
        ):
            # ---- input DMAs, program order == ring FIFO order ----
            # First Gram chunks lead on both rings so the PE starts ASAP;
            # weights ride the sync ring and xc(batch 0) the scalar ring a
            # couple of chunks in (both are only needed mid-stream).
            xg_tiles = [[], []]

            def make_xg(b, c):
                t = data.tile([P, CHW], FP8, tag=f"xg{b}_{c}")
                eng = nc.sync if c % 2 == 0 else nc.scalar
                off = (b * SUB + c * CHUNK_SUB) * P
                eng.dma_start(t[:], xg_ext[:, off : off + CHW])
                xg_tiles[b].append(t)

            for c in range(2):
                make_xg(0, c)

            wpack = const.tile([P, WCOLS], BF, tag="wpack")
            nc.sync.dma_start(wpack[:], w_ext[:])
            wq = wpack[:, 0:512]
            wk = wpack[:, 512:1024]
            wv = wpack[:, 1024:1536]
            wo = wpack[:, 1536:1792]

            xc = data.tile([P, 2 * NSH], BF, tag="xc")
            nc.scalar.dma_start(xc[:, 0:NSH], xc_ext[:, 0:NSH])

            for c in range(2, NCHUNK):
                make_xg(0, c)
            for c in range(NCHUNK):
                make_xg(1, c)

            nc.sync.dma_start(
                xc[:, NSH : NSH + NSH // 2], xc_ext[:, NSH : NSH + NSH // 2]
            )
            nc.scalar.dma_start(
                xc[:, NSH + NSH // 2 :], xc_ext[:, NSH + NSH // 2 :]
            )

            # f32 bias + dummy source for the HAM warm-keepers.  zero1/one1
            # replace float immediates in activation(): the framework's
            # const-AP tensors are written by an init-time gpsimd memset
            # that is only safe behind the init barrier we skip above.
            bo = work.tile([P, 1], F32, tag="bo")
            dummy_src = work.tile([P, 512], F32, tag="dummy")
            nc.vector.memset(dummy_src[:], 0.0)
            zero1 = work.tile([P, 1], F32, tag="zero1")
            nc.vector.memset(zero1[:], 0.0)
            one1 = work.tile([P, 1], F32, tag="one1")
            nc.vector.memset(one1[:], 1.0)

            # ---- HAM warm-keepers: PE busy through the DMA prefill ----
            warm_ps = psw.tile([P, 512], F32, tag="warm")
            for w in range(WARM_MMS):
                nc.tensor.matmul(
                    warm_ps[:], dummy_src[:, :P], dummy_src[:],
                    start=True, stop=True,
                )

            nc.vector.tensor_copy(bo[:], wpack[:, 1792:1793])

            # ---- Gram accumulation (fp8 DoubleRow) ----
            g_ps = [None, None]
            gbf = [None, None]

            def gram_chunks(b, c_lo, c_hi):
                if g_ps[b] is None:
                    g_ps[b] = psg.tile([P, P], F32, tag="g", name=f"g_ps{b}")
                n_mm = CHUNK_SUB // 2
                for c in range(c_lo, c_hi):
                    xr = xg_tiles[b][c][:].rearrange("p (m c) -> p m c", c=P)
                    for j in range(n_mm):
                        sl = xr[:, 2 * j : 2 * j + 2, :]
                        nc.tensor.matmul(
                            g_ps[b][:], sl, sl,
                            start=(c == 0 and j == 0),
                            stop=(c == NCHUNK - 1 and j == n_mm - 1),
                            perf_mode=DR,
                        )
                if c_hi == NCHUNK:
                    gbf[b] = work.tile([P, P], BF, tag=f"gbf{b}", name=f"gbf{b}")
                    nc.vector.tensor_copy(gbf[b][:], g_ps[b][:])

            # ---- phase D split into PE-segments (chain runs off-PE) ----
            def phase_d_scores(b):
                """a = G Wq, S = a^T Wk -- PE part one."""
                a_ps = psd.tile([P, 512], F32, tag="d", name=f"a_ps{b}")
                a_sb = work.tile([P, 512], BF, tag=f"asb{b}", name=f"a_sb{b}")
                s_ps = psd.tile([P, 256], F32, tag="d", name=f"s_ps{b}")
                nc.tensor.matmul(a_ps[:], gbf[b][:], wq, start=True, stop=True)
                for sl in range(4):
                    nc.vector.tensor_copy(
                        a_sb[:, sl * 128 : (sl + 1) * 128],
                        a_ps[:, sl * 128 : (sl + 1) * 128],
                    )
                for h in range(HEADS):
                    pb = 64 * (h % 2)
                    cg = 64 * (h // 2)
                    nc.tensor.matmul(
                        s_ps[pb : pb + 64, cg : cg + 64],
                        a_sb[:, h * 64 : (h + 1) * 64],
                        wk[:, h * 64 : (h + 1) * 64],
                        start=True, stop=True,
                    )
                return s_ps

            def phase_d_softmax(b, s_ps):
                """softmax over each head group -- DVE/ACT only."""
                negmax = work.tile([P, 4], F32, tag=f"nm{b}", name=f"negmax{b}")
                sm_sb = work.tile([P, 256], F32, tag=f"sm{b}", name=f"sm_sb{b}")
                exp_sb = work.tile([P, 256], F32, tag=f"exp{b}", name=f"exp_sb{b}")
                sums = work.tile([P, 4], F32, tag=f"sums{b}", name=f"sums{b}")
                recip = work.tile([P, 4], F32, tag=f"recip{b}", name=f"recip{b}")
                attn = work.tile([P, 256], BF, tag=f"attn{b}", name=f"attn{b}")
                nc.vector.reduce_max(
                    negmax[:],
                    s_ps[:].rearrange("p (g j) -> p g j", j=64),
                    axis=mybir.AxisListType.X,
                    negate=True,
                )
                nc.vector.tensor_tensor(
                    sm_sb[:].rearrange("p (g j) -> p g j", j=64),
                    s_ps[:].rearrange("p (g j) -> p g j", j=64),
                    negmax[:].rearrange("p g -> p g ()").broadcast_to((P, 4, 64)),
                    op=mybir.AluOpType.add,
                )
                nc.scalar.activation(
                    exp_sb[:],
                    sm_sb[:],
                    mybir.ActivationFunctionType.Exp,
                    bias=zero1[:, 0:1],
                    scale=one1[:, 0:1],
                )
                nc.vector.reduce_sum(
                    sums[:],
                    exp_sb[:].rearrange("p (g j) -> p g j", j=64),
                    axis=mybir.AxisListType.X,
                )
                nc.vector.reciprocal(recip[:], sums[:])
                nc.vector.tensor_tensor(
                    attn[:].rearrange("p (g j) -> p g j", j=64),
                    exp_sb[:].rearrange("p (g j) -> p g j", j=64),
                    recip[:].rearrange("p g -> p g ()").broadcast_to((P, 4, 64)),
                    op=mybir.AluOpType.mult,
                )
                return attn

            def phase_d_weff(b, attn):
                """MT = attn^T WoT, W_eff = wv MT -- PE part two."""
                mt_ps = psd.tile([P, 256], F32, tag="d", name=f"mt_ps{b}")
                mt_sb = work.tile([P, 256], BF, tag=f"mt{b}", name=f"mt_sb{b}")
                w_ps = psd.tile([P, 64], F32, tag="d", name=f"w_ps{b}")
                weff = work.tile([P, 64], BF, tag=f"weff{b}", name=f"weff{b}")
                for h in range(HEADS):
                    pb = 64 * (h % 2)
                    cg = 64 * (h // 2)
                    nc.tensor.matmul(
                        mt_ps[pb : pb + 64, cg : cg + 64],
                        attn[pb : pb + 64, cg : cg + 64],
                        wo[pb : pb + 64, cg : cg + 64],
                        start=True, stop=True,
                    )
                nc.vector.tensor_copy(mt_sb[:], mt_ps[:])
                for g in range(4):
                    nc.tensor.matmul(
                        w_ps[:],
                        wv[:, g * P : (g + 1) * P],
                        mt_sb[:, g * 64 : (g + 1) * 64],
                        start=(g == 0), stop=(g == 3),
                    )
                nc.vector.tensor_copy(weff[:], w_ps[:])
                return weff

            def phase_e(b, weff, out_swdge):
                """y_b = W_eff_b @ x_b + b_out, chunked + streamed out."""
                for j in range(8):
                    y_ps = psy.tile([64, 512], F32, tag="y", name=f"y_ps{b}_{j}")
                    nc.tensor.matmul(
                        y_ps[:],
                        weff[:],
                        xc[:, b * NSH + j * 512 : b * NSH + (j + 1) * 512],
                        start=True, stop=True,
                    )
                    y_sb = ypool.tile(
                        [64, 512], BF, tag="ysb", name=f"y_sb{b}_{j}"
                    )
                    nc.any.tensor_scalar_add(y_sb[:], y_ps[:], bo[0:64, 0:1])
                    dst = out_ext[b * 64 : (b + 1) * 64, j * 512 : (j + 1) * 512]
                    if out_swdge:
                        # HWDGE rings are busy with the input-stream FIFO;
                        # ship batch 0's output on the SWDGE ring instead.
                        nc.gpsimd.dma_start(dst, y_sb[:])
                    else:
                        eng = nc.sync if j % 2 == 0 else nc.scalar
                        eng.dma_start(dst, y_sb[:])

            # ---- PE program order ----
            gram_chunks(0, 0, NCHUNK)
            s_ps0 = phase_d_scores(0)
            gram_chunks(1, 0, 2)                 # softmax0 runs under these
            attn0 = phase_d_softmax(0, s_ps0)    # (DVE/ACT ops, off the PE)
            gram_chunks(1, 2, 4)
            weff0 = phase_d_weff(0, attn0)
            gram_chunks(1, 4, 6)
            phase_e(0, weff0, out_swdge=True)
            gram_chunks(1, 6, NCHUNK)
            s_ps1 = phase_d_scores(1)
            attn1 = phase_d_softmax(1, s_ps1)
            weff1 = phase_d_weff(1, attn1)
            phase_e(1, weff1, out_swdge=False)

    nc.compile()
    return nc


def _get_nc():
    global _CACHED_NC
    if _CACHED_NC is None:
        _CACHED_NC = build_nc()
    return _CACHED_NC


def make_in_maps(x, w_qkv, w_out, b_out):
    x = np.ascontiguousarray(x, dtype=np.float32)
    w_qkv = np.asarray(w_qkv, dtype=np.float32)
    w_out = np.asarray(w_out, dtype=np.float32)
    b_out = np.asarray(b_out, dtype=np.float32)
    xf = x.reshape(2, P, N_TOT)

    # full x, fp8, DoubleRow layout [p, b, m, c]
    xg_h = np.ascontiguousarray(
        xf.transpose(0, 2, 1)            # (2, n, c)
        .reshape(2, SUB, P, P)           # (2, m, p, c)
        .transpose(2, 0, 1, 3)           # (p, 2, m, c)
        .reshape(P, 2 * SUB * P)
    ).astype(f8)

    wpack = np.zeros((P, WCOLS), np.float32)
    wpack[:, 0:512] = w_qkv[:512].T * SCALE
    wpack[:, 512:1024] = w_qkv[512:1024].T
    wpack[:, 1024:1536] = (
        (w_qkv[1024:] / N_TOT).reshape(4, P, P).transpose(1, 0, 2).reshape(P, 512)
    )
    for h in range(HEADS):
        wpack[
            64 * (h % 2) : 64 * (h % 2) + 64,
            1536 + 64 * (h // 2) : 1536 + 64 * (h // 2) + 64,
        ] = w_out[:, h * 64 : (h + 1) * 64].T
    wpack[:, 1792] = np.concatenate([b_out, b_out])
    wpack_h = wpack.astype(bf16)

    in_maps = []
    for c in range(NCORES):
        # own output shard, bf16, [c, (b, n)]
        xc_h = np.ascontiguousarray(
            xf[:, :, c * NSH : (c + 1) * NSH].transpose(1, 0, 2).reshape(P, 2 * NSH)
        ).astype(bf16)
        in_maps.append({"xg": xg_h, "xc": xc_h, "wpack": wpack_h})
    return in_maps


def assemble_output(results):
    y = np.empty((2, 64, N_TOT), np.float32)
    for c in range(NCORES):
        o = np.asarray(results[c]["out"]).astype(np.float32)  # [128, 4096] bf16
        y[0, :, c * NSH : (c + 1) * NSH] = o[:64]
        y[1, :, c * NSH : (c + 1) * NSH] = o[64:]
    return y.reshape(2, 64, 32, 32, 32)


def kernel(**inputs):
    in_maps = make_in_maps(
        inputs["x"], inputs["w_qkv"], inputs["w_out"], inputs["b_out"]
    )
    nc = _get_nc()
    res = run_bass_kernel_spmd(nc, in_maps, core_ids=list(range(NCORES)))
    return assemble_output(res.results)
